# revision 1
# baseline (speedup 1.0000x reference)
"""InternLM3 self-attention (prefill, GQA, RoPE) on 8 Trainium2 cores.

Tensor-parallel over heads: core r owns q heads 4r..4r+3 and kv head r
(wqkv column shards, wo row shards).  Each core computes its partial
output projection; the 8 partials are summed on the host (an on-device
all-reduce of 32 MB runs at ~32 GB/s through ncfw and would dominate the
kernel, so the reduction is done host-side).

Matmuls run in float32r (TF32-like fast fp32 mode, 1 cycle/row at
N>=512 vs 4 for plain fp32) with fp32 PSUM accumulation.

Device-side layout trick: everything is computed transposed
(qkv^T = wqkv_shard^T @ hidden^T) so that
  - wqkv loads land directly as the stationary operand,
  - q^T/k^T slices feed the scores matmul with head_dim on partitions,
  - scores come out as S^T [k, q], so exp(S^T) feeds the PV matmul
    directly (contraction over k on partitions) with zero transposes,
  - attn^T slices are exactly the stationary operand of the wo matmul.
The only transposes are hidden^T (done host-side, it is an input-layout
choice) and v^T -> v (16 tiny PE transposes).
"""

import numpy as np

import concourse.bass as bass
import concourse.bacc as bacc
import concourse.mybir as mybir
import concourse.tile as tile
from concourse.bass_utils import run_bass_kernel_spmd

T = 2048
H = 4096
NH = 32
NKV = 8
HD = 128
HALF = HD // 2
BASE = 1000000.0
NCORES = 8
QH = NH // NCORES            # 4 q heads per core
QCOLS = QH * HD              # 512
SH_COLS = QCOLS + 2 * HD     # 768 wqkv cols per core
NEG = -1e30

P = 128
TC = 512                     # token chunk (matmul moving dim)
NT = T // TC                 # 4
NHC = H // P                 # 32 contraction chunks for qkv
NQC = SH_COLS // P           # 6 qkv col chunks
NKC = T // P                 # 16 k chunks
NOC = H // TC                # 8 output col chunks
NTC16 = T // P               # 16 token chunks of 128

f32 = mybir.dt.float32
f32r = mybir.dt.float32r

_COMPILED = None


def _build():
    nc = bacc.Bacc("TRN2", target_bir_lowering=False, debug=False,
                   num_devices=NCORES)

    hidT = nc.dram_tensor("hidT", [H, T], f32r, kind="ExternalInput").ap()
    wqkv_s = nc.dram_tensor("wqkv_s", [H, SH_COLS], f32r,
                            kind="ExternalInput").ap()
    wo_s = nc.dram_tensor("wo_s", [QCOLS, H], f32r,
                          kind="ExternalInput").ap()
    cosq = nc.dram_tensor("cosq", [P, T], f32, kind="ExternalInput").ap()
    sinq = nc.dram_tensor("sinq", [P, T], f32, kind="ExternalInput").ap()
    cosk = nc.dram_tensor("cosk", [P, T], f32, kind="ExternalInput").ap()
    sink = nc.dram_tensor("sink", [P, T], f32, kind="ExternalInput").ap()
    masks = nc.dram_tensor("masks", [P, 4, TC], f32,
                           kind="ExternalInput").ap()
    rperm = nc.dram_tensor("rperm", [P, P], f32r, kind="ExternalInput").ap()
    ident = nc.dram_tensor("ident", [P, P], f32r, kind="ExternalInput").ap()
    ones_k = nc.dram_tensor("ones_k", [P, 1], f32r,
                            kind="ExternalInput").ap()
    ones_m = nc.dram_tensor("ones_m", [1, P], f32r,
                            kind="ExternalInput").ap()
    part = nc.dram_tensor("part", [T, H], f32, kind="ExternalOutput").ap()

    with tile.TileContext(nc) as tc:
        with tc.tile_pool(name="keep", bufs=1) as keep:
            # long-lived SBUF: qkv^T [128, 6, 2048] f32r (48 KB/part)
            qkvT = keep.tile([P, NQC, T], f32r)

            # constants first: tiny DMAs, land before the bulk loads
            ct = keep.tile([P, T], f32, tag="cosq_t")
            st = keep.tile([P, T], f32, tag="sinq_t")
            ctk = keep.tile([P, T], f32, tag="cosk_t")
            stk = keep.tile([P, T], f32, tag="sink_t")
            mt = keep.tile([P, 4, TC], f32, tag="masks_t")
            rp = keep.tile([P, P], f32r, tag="rperm_t")
            idt = keep.tile([P, P], f32r, tag="ident_t")
            o_k = keep.tile([P, 1], f32r, tag="ones_k_t")
            o_m = keep.tile([1, P], f32r, tag="ones_m_t")

            # ---------------- phase 1: qkv^T = wqkv^T @ hidden^T -------
            with tc.tile_pool(name="wq", bufs=1) as wqp, \
                 tc.tile_pool(name="hstream", bufs=4) as hsp, \
                 tc.tile_pool(name="qps", bufs=1, space="PSUM") as qpsp:
                wq = wqp.tile([P, NHC, SH_COLS], f32r)
                for h in range(NHC):
                    nc.sync.dma_start(
                        wq[:, h, :], wqkv_s[h * P:(h + 1) * P, :])
                nc.sync.dma_start(ct[:], cosq[:])
                nc.sync.dma_start(st[:], sinq[:])
                nc.sync.dma_start(ctk[:], cosk[:])
                nc.sync.dma_start(stk[:], sink[:])
                nc.sync.dma_start(mt[:], masks[:])
                nc.sync.dma_start(rp[:], rperm[:])
                nc.sync.dma_start(idt[:], ident[:])
                nc.sync.dma_start(o_k[:], ones_k[:])
                nc.sync.dma_start(o_m[:], ones_m[:])
                for t in range(NT):
                    qps = [qpsp.tile([P, TC], f32, tag=f"qps{c}",
                                     name=f"qps{c}_{t}")
                           for c in range(NQC)]
                    for h in range(NHC):
                        ht = hsp.tile([P, TC], f32r, tag="ht")
                        nc.scalar.dma_start(
                            ht[:], hidT[h * P:(h + 1) * P,
                                        t * TC:(t + 1) * TC])
                        for c in range(NQC):
                            nc.tensor.matmul(
                                qps[c][:], wq[:, h, c * P:(c + 1) * P],
                                ht[:], start=(h == 0), stop=(h == NHC - 1))
                    for c in range(NQC):
                        nc.scalar.copy(
                            qkvT[:, c, t * TC:(t + 1) * TC], qps[c][:])

            with tc.tile_pool(name="keep2", bufs=1) as keep2:
                    # ---------------- phase 3: v_nat = v^T transposed ----------
                vnat = keep2.tile([P, NKC, P], f32r, tag="vnat")
                with tc.tile_pool(name="vt_ps", bufs=4, space="PSUM") as vps:
                    for kc in range(NKC):
                        tp = vps.tile([P, P], f32r, tag="vtp")
                        nc.tensor.transpose(
                            tp[:], qkvT[:, 5, kc * P:(kc + 1) * P], idt[:])
                        nc.scalar.copy(vnat[:, kc, :], tp[:])

                # ---------------- phase 2: RoPE on q (scaled) and k --------
                with tc.tile_pool(name="rope_sb", bufs=4) as rsb, \
                     tc.tile_pool(name="rope_ps", bufs=4, space="PSUM") as rps:
                    for idx in range(QH + 1):        # 4 q heads + 1 k head
                        cos_t, sin_t = (ct, st) if idx < QH else (ctk, stk)
                        for t in range(NT):
                            sl = slice(t * TC, (t + 1) * TC)
                            x = qkvT[:, idx, sl]
                            rot = rps.tile([P, TC], f32, tag="rot")
                            nc.tensor.matmul(rot[:], rp[:], x,
                                             start=True, stop=True)
                            tmp = rsb.tile([P, TC], f32, tag="rtmp")
                            nc.vector.tensor_tensor(
                                tmp[:], rot[:], sin_t[:, sl],
                                mybir.AluOpType.mult)
                            nc.vector.tensor_tensor(
                                x, x.bitcast(f32), cos_t[:, sl],
                                mybir.AluOpType.mult)
                            nc.vector.tensor_tensor(
                                x, x.bitcast(f32), tmp[:],
                                mybir.AluOpType.add)

                # ---------------- phase 4: causal attention ----------------
                attnT = keep2.tile([P, QH, T], f32r, tag="attnT")
                with tc.tile_pool(name="att_sb", bufs=8) as asb, \
                     tc.tile_pool(name="att_sm", bufs=4) as asm_p, \
                     tc.tile_pool(name="st_ps", bufs=3, space="PSUM") as stp, \
                     tc.tile_pool(name="pv_ps", bufs=2, space="PSUM") as pvp, \
                     tc.tile_pool(name="d_ps", bufs=2, space="PSUM") as dpp, \
                     tc.tile_pool(name="rb_ps", bufs=1, space="PSUM") as rbp:
                    for head in range(QH):
                        for g in range(NT):
                            kmax = (NT // 1) * (g + 1)   # 4*(g+1) k chunks
                            qsl = slice(g * TC, (g + 1) * TC)
                            d_ps = dpp.tile([1, TC], f32, tag="d")
                            pv = pvp.tile([P, TC], f32, tag="pv")
                            es = asb.tile([P, TC], f32r, tag="esum")
                            e_prev = None
                            for kc in range(kmax):
                                st_ps = stp.tile([P, TC], f32, tag="st")
                                nc.tensor.matmul(
                                    st_ps[:],
                                    qkvT[:, QH, kc * P:(kc + 1) * P],
                                    qkvT[:, head, qsl],
                                    start=True, stop=True)
                                j = kc - 4 * g
                                if j >= 0:
                                    nc.vector.tensor_tensor(
                                        st_ps[:], st_ps[:], mt[:, j, :],
                                        mybir.AluOpType.add)
                                e = asb.tile([P, TC], f32r, tag="E",
                                             name=f"e_{head}_{g}_{kc}")
                                nc.scalar.activation(
                                    e[:], st_ps[:],
                                    mybir.ActivationFunctionType.Exp)
                                # denominator partials on DVE (frees PE)
                                if kc == 1:
                                    nc.vector.tensor_tensor(
                                        es[:], e_prev[:], e[:],
                                        mybir.AluOpType.add)
                                elif kc > 1:
                                    nc.vector.tensor_tensor(
                                        es[:], es[:], e[:],
                                        mybir.AluOpType.add)
                                e_prev = e
                                nc.tensor.matmul(
                                    pv[:], vnat[:, kc, :], e[:],
                                    start=(kc == 0), stop=(kc == kmax - 1))
                            nc.tensor.matmul(d_ps[:], o_k[:], es[:],
                                             start=True, stop=True)
                            rd = asm_p.tile([1, TC], f32, tag="rd")
                            nc.vector.reciprocal(rd[:], d_ps[:])
                            rdr = asm_p.tile([1, TC], f32r, tag="rdr")
                            nc.scalar.copy(rdr[:], rd[:])
                            rb = rbp.tile([P, TC], f32, tag="rb")
                            nc.tensor.matmul(rb[:], o_m[:], rdr[:],
                                             start=True, stop=True)
                            rbs = asm_p.tile([P, TC], f32, tag="rbs")
                            nc.scalar.copy(rbs[:], rb[:])
                            nc.vector.tensor_tensor(
                                attnT[:, head, qsl], pv[:], rbs[:],
                                mybir.AluOpType.mult)

                # ---------------- phase 5: out = attn @ wo_shard -----------
                with tc.tile_pool(name="wo_sb", bufs=3) as wsb, \
                     tc.tile_pool(name="o_sb", bufs=4) as osb, \
                     tc.tile_pool(name="o_ps", bufs=4, space="PSUM") as ops:
                    for oc in range(NOC):
                        wot = wsb.tile([P, QH, TC], f32r, tag="wot")
                        nc.sync.dma_start(
                            wot[:],
                            wo_s[:, oc * TC:(oc + 1) * TC].rearrange(
                                "(hc p) n -> p hc n", p=P))
                        for tcn in range(NTC16):
                            o_ps = ops.tile([P, TC], f32, tag="o")
                            for hc in range(QH):
                                nc.tensor.matmul(
                                    o_ps[:],
                                    attnT[:, hc, tcn * P:(tcn + 1) * P],
                                    wot[:, hc, :],
                                    start=(hc == 0), stop=(hc == QH - 1))
                            ob = osb.tile([P, TC], f32, tag="ob")
                            nc.scalar.copy(ob[:], o_ps[:])
                            nc.gpsimd.dma_start(
                                part[tcn * P:(tcn + 1) * P,
                                     oc * TC:(oc + 1) * TC], ob[:])

    nc.compile()
    return nc


def _rope_tables(positions):
    pos = positions.astype(np.float64)
    inv_freq = 1.0 / (BASE ** (np.arange(HALF, dtype=np.float64) / HALF))
    freqs = pos[:, None] * inv_freq[None, :]          # [T, 64]
    cos = np.cos(freqs)
    sin = np.sin(freqs)
    cosT = np.concatenate([cos, cos], axis=1).T       # [128, T]
    sinT = np.concatenate([-sin, sin], axis=1).T      # sign folded
    return cosT.astype(np.float32), sinT.astype(np.float32)


def kernel(positions, hidden_states, wqkv, wo):
    global _COMPILED
    if _COMPILED is None:
        _COMPILED = _build()
    nc = _COMPILED

    scale = HD ** -0.5
    cosT, sinT = _rope_tables(positions)
    cosq = np.ascontiguousarray(cosT * scale)
    sinq = np.ascontiguousarray(sinT * scale)

    hidT = np.ascontiguousarray(hidden_states.T)

    # causal mask add-tiles for the diagonal blocks, ST layout [k, q]:
    # block j (k chunk 4g+j vs q group g): valid iff 128*j + kl <= ql
    kl = np.arange(P)[:, None]
    ql = np.arange(TC)[None, :]
    masks = np.stack(
        [np.where(P * j + kl <= ql, 0.0, NEG) for j in range(4)],
        axis=1).astype(np.float32)                    # [128, 4, 512]

    rperm = np.zeros((P, P), dtype=np.float32)
    for m in range(P):
        rperm[(m + HALF) % P, m] = 1.0                # out[m]=x[(m+64)%128]
    ident = np.eye(P, dtype=np.float32)
    ones_k = np.ones((P, 1), dtype=np.float32)
    ones_m = np.ones((1, P), dtype=np.float32)

    in_maps = []
    for r in range(NCORES):
        qc = slice(r * QCOLS, (r + 1) * QCOLS)
        kc = slice(NH * HD + r * HD, NH * HD + (r + 1) * HD)
        vc = slice((NH + NKV) * HD + r * HD, (NH + NKV) * HD + (r + 1) * HD)
        wqkv_s = np.ascontiguousarray(
            np.concatenate([wqkv[:, qc], wqkv[:, kc], wqkv[:, vc]], axis=1))
        wo_s = np.ascontiguousarray(wo[qc, :])
        in_maps.append({
            "hidT": hidT, "wqkv_s": wqkv_s, "wo_s": wo_s,
            "cosq": cosq, "sinq": sinq, "cosk": cosT, "sink": sinT,
            "masks": masks, "rperm": rperm, "ident": ident,
            "ones_k": ones_k, "ones_m": ones_m,
        })

    global _LAST_IN_MAPS
    _LAST_IN_MAPS = in_maps
    res = run_bass_kernel_spmd(nc, in_maps, list(range(NCORES)))
    out = res.results[0]["part"].astype(np.float64)
    for r in range(1, NCORES):
        out += res.results[r]["part"]
    return out.astype(np.float32)



# revision 10
# speedup vs baseline: 1.0801x; 1.0801x over previous
"""InternLM3 self-attention (prefill, GQA, RoPE) on 8 Trainium2 cores.

Tensor-parallel over heads: core r owns q heads 4r..4r+3 and kv head r
(wqkv column shards, wo row shards).  Each core computes its partial
output projection in bf16; the 8 partials are summed on the host.

v2 design (vs the fp32r v1 baseline at ~640-700us):
  - every matmul in bf16 (FWL auto-enables, LDWEIGHTS fully hidden,
    1 cycle/row always; fp32 PSUM accumulation keeps precision).
  - softmax denominator for free: v is stored with an appended ones
    column, so the PV matmul accumulates sum(e) in PSUM column 128.
  - PV computed transposed (out [q, hd]) so the 1/d normalization is a
    per-partition tensor_scalar with a [128,1] reciprocal (the v1
    [1,512] DVE reciprocals cost 53us); the normalized tile is moved
    into WO layout with a free DMA-XBAR transpose.
  - causal trimming at 128 granularity (diagonal 512-blocks stepped).
  - RoPE rotate-half via two partition-offset SBUF DMAs (no PE).
  - v projected directly in natural [tok, hd] layout by swapping
    stationary/moving operands (no PE transposes at all).
  - software pipelining: scores(kc) stream ahead of exp (scalar) with
    PV one phase behind; WO groups of the previous q-block interleave
    into the exp-bound attention stretches to keep PE busy.
"""

import numpy as np
from ml_dtypes import bfloat16

import concourse.bass as bass
import concourse.bacc as bacc
import concourse.mybir as mybir
import concourse.tile as tile
from concourse.bass_utils import run_bass_kernel_spmd

T = 2048
H = 4096
NH = 32
NKV = 8
HD = 128
HALF = HD // 2
BASE = 1000000.0
NCORES = 8
QH = NH // NCORES            # 4 q heads per core
QCOLS = QH * HD              # 512
NEG = -1e30

P = 128
TC = 256                     # qkv-projection token chunk
NT = T // TC                 # 8
NHC = H // P                 # 32 contraction chunks
G = 512                      # attention q block
NG = T // G                  # 4
NKC = T // P                 # 16 k chunks of 128

f32 = mybir.dt.float32
bf16 = mybir.dt.bfloat16

_COMPILED = None
DEBUG_DUMP = False


def _build():
    nc = bacc.Bacc("TRN2", target_bir_lowering=False, debug=False,
                   num_devices=NCORES)

    hidT = nc.dram_tensor("hidT", [H, T], bf16, kind="ExternalInput").ap()
    wq_d = nc.dram_tensor("wq_d", [H, 768], bf16, kind="ExternalInput").ap()
    wo_d = nc.dram_tensor("wo_d", [P, QH, H], bf16,
                          kind="ExternalInput").ap()
    cosq = nc.dram_tensor("cosq", [P, T], bf16, kind="ExternalInput").ap()
    sinq = nc.dram_tensor("sinq", [P, T], bf16, kind="ExternalInput").ap()
    cosk = nc.dram_tensor("cosk", [P, T], bf16, kind="ExternalInput").ap()
    sink = nc.dram_tensor("sink", [P, T], bf16, kind="ExternalInput").ap()
    maskd = nc.dram_tensor("maskd", [P, P], f32, kind="ExternalInput").ap()
    part = nc.dram_tensor("part", [T, H], bf16, kind="ExternalOutput").ap()
    if DEBUG_DUMP:
        dbg_qkT = nc.dram_tensor("dbg_qkT", [P, 5, T], bf16,
                                 kind="ExternalOutput").ap()
        dbg_vnat = nc.dram_tensor("dbg_vnat", [P, NKC, 132], bf16,
                                  kind="ExternalOutput").ap()
        dbg_attnT = nc.dram_tensor("dbg_attnT", [P, QH, T], bf16,
                                   kind="ExternalOutput").ap()

    with tile.TileContext(nc) as tc:
        with tc.tile_pool(name="keep", bufs=1) as keep, \
             tc.tile_pool(name="hstream", bufs=36) as hsp, \
             tc.tile_pool(name="xfp", bufs=8) as xfp, \
             tc.tile_pool(name="rotp", bufs=4) as rotp, \
             tc.tile_pool(name="t12p", bufs=4) as t12p, \
             tc.tile_pool(name="ep", bufs=18) as ep, \
             tc.tile_pool(name="rdp", bufs=4) as rdp, \
             tc.tile_pool(name="pvsbp", bufs=4) as pvsbp, \
             tc.tile_pool(name="outp", bufs=6) as outp, \
             tc.tile_pool(name="aps", bufs=2, space="PSUM") as aps, \
             tc.tile_pool(name="stps", bufs=2, space="PSUM") as stps, \
             tc.tile_pool(name="wops", bufs=2, space="PSUM") as wops, \
             tc.tile_pool(name="pvps", bufs=2, space="PSUM") as pvps:

            # ---------------- long-lived SBUF ----------------
            wq = keep.tile([P, NHC, 768], bf16, tag="wq_t")
            qkT = keep.tile([P, 5, T], bf16, tag="qkT_t")
            vnat = keep.tile([P, NKC, 132], bf16, tag="vnat_t")
            attnT = keep.tile([P, QH, T], bf16, tag="attnT_t")
            wot = keep.tile([P, QH, H], bf16, tag="wot_t")
            ctq = keep.tile([P, T], bf16, tag="cosq_t")
            stq = keep.tile([P, T], bf16, tag="sinq_t")
            ctk = keep.tile([P, T], bf16, tag="cosk_t")
            stk = keep.tile([P, T], bf16, tag="sink_t")
            mt = keep.tile([P, P], f32, tag="mask_t")

            # constants + weights first (small first so they land early)
            nc.sync.dma_start(mt[:], maskd[:])
            nc.sync.dma_start(ctq[:], cosq[:])
            nc.sync.dma_start(stq[:], sinq[:])
            nc.sync.dma_start(ctk[:], cosk[:])
            nc.sync.dma_start(stk[:], sink[:])
            nc.vector.memset(vnat[:, :, 128:129], 1.0)
            for h in range(NHC):
                nc.sync.dma_start(wq[:, h, :], wq_d[h * P:(h + 1) * P, :])
            for hc in range(QH):
                nc.scalar.dma_start(wot[:, hc, :], wo_d[:, hc, :])

            # WO groups of q-block g, interleaved into attention of g+1
            wo_queue = []

            def emit_wo_group():
                tcn, oc = wo_queue.pop(0)
                o_ps = wops.tile([P, G], f32, tag="wo",
                                 name=f"o_{tcn}_{oc}")
                for hc in range(QH):
                    nc.tensor.matmul(
                        o_ps[:], attnT[:, hc, tcn * P:(tcn + 1) * P],
                        wot[:, hc, oc * G:(oc + 1) * G],
                        start=(hc == 0), stop=(hc == QH - 1))
                ob = outp.tile([P, G], bf16, tag="ob",
                               name=f"ob_{tcn}_{oc}")
                if (tcn + oc) % 2 == 0:
                    nc.scalar.copy(ob[:], o_ps[:])
                else:
                    nc.vector.tensor_scalar_add(ob[:], o_ps[:], 0.0)
                nc.gpsimd.dma_start(
                    part[tcn * P:(tcn + 1) * P, oc * G:(oc + 1) * G], ob[:])

            def rope(c, t, xf):
                # qkT[:, c, t*TC:+TC] = xf*cos + rot(xf)*sin
                cos_t, sin_t = (ctq, stq) if c < QH else (ctk, stk)
                sl = slice(t * TC, (t + 1) * TC)
                rot = rotp.tile([P, TC], bf16, tag="rot",
                                name=f"rot_{c}_{t}")
                nc.sync.dma_start(rot[0:HALF, :], xf[HALF:P, :])
                nc.sync.dma_start(rot[HALF:P, :], xf[0:HALF, :])
                t1 = t12p.tile([P, TC], f32, tag="t12", name=f"t1_{c}_{t}")
                t2 = t12p.tile([P, TC], f32, tag="t12", name=f"t2_{c}_{t}")
                nc.vector.tensor_tensor(t1[:], xf[:], cos_t[:, sl],
                                        mybir.AluOpType.mult)
                nc.vector.tensor_tensor(t2[:], rot[:], sin_t[:, sl],
                                        mybir.AluOpType.mult)
                nc.vector.tensor_tensor(qkT[:, c, sl], t1[:], t2[:],
                                        mybir.AluOpType.add)

            def attn_block(g):
                """Attention for q-block g; interleaves WO of block g-1.

                PV runs as two sweeps (s-pairs {0,1} then {2,3}); the two
                concurrently-accumulating groups of a sweep sit in separate
                full-bank PSUM tiles (start=True clears the whole bank's
                has_written bits, so interleaved groups must not share).
                """
                for head in range(QH):
                    kmax = 4 * (g + 1)
                    e_tiles = []
                    for kc in range(kmax):
                        j = kc - 4 * g
                        W = G if j < 0 else G - P * j
                        qo = g * G + (G - W)
                        stt = stps.tile([P, G], f32, tag="st",
                                        name=f"st_{g}_{head}_{kc}")
                        nc.tensor.matmul(
                            stt[:, 0:W],
                            qkT[:, QH, kc * P:(kc + 1) * P],
                            qkT[:, head, qo:qo + W],
                            start=True, stop=True)
                        if j >= 0:
                            nc.vector.tensor_tensor(
                                stt[:, 0:P], stt[:, 0:P], mt[:],
                                mybir.AluOpType.add)
                        e = ep.tile([P, G], bf16, tag="e",
                                    name=f"e_{g}_{head}_{kc}")
                        nc.scalar.activation(
                            e[:, 0:W], stt[:, 0:W],
                            mybir.ActivationFunctionType.Exp)
                        e_tiles.append((kc, W, e))
                    # PV + normalize, with WO groups interleaved
                    wo_done = [0]
                    steps = [0]
                    total_steps = 2 * kmax

                    def maybe_wo():
                        steps[0] += 1
                        want = steps[0] * 8 // total_steps
                        while wo_queue and wo_done[0] < want:
                            emit_wo_group()
                            wo_done[0] += 1

                    rd = rdp.tile([P, 4], f32, tag="rd",
                                  name=f"rd_{g}_{head}")
                    for sp in range(2):           # s pairs {0,1}, {2,3}
                        pvt = [pvps.tile([P, G], f32, tag="pv",
                                         name=f"pv_{g}_{head}_{sp}_{i}")
                               for i in range(2)]
                        for kc, W, e in e_tiles:
                            j = kc - 4 * g
                            for i in range(2):
                                s = 2 * sp + i
                                if j > s or kc > 4 * g + s:
                                    continue
                                eoff = s * P - (G - W)
                                nc.tensor.matmul(
                                    pvt[i][:, 0:129],
                                    e[:, eoff:eoff + P],
                                    vnat[:, kc, 0:129],
                                    start=(kc == 0), stop=(kc == 4 * g + s))
                            maybe_wo()
                        for i in range(2):
                            s = 2 * sp + i
                            nc.vector.reciprocal(rd[:, s:s + 1],
                                                 pvt[i][:, 128:129])
                            pvn = pvsbp.tile([P, P], bf16, tag="pvn",
                                             name=f"pvn_{g}_{head}_{s}")
                            nc.vector.tensor_scalar(
                                pvn[:], pvt[i][:, 0:P], rd[:, s:s + 1],
                                None, mybir.AluOpType.mult)
                            tsl = (4 * g + s) * P
                            nc.sync.dma_start_transpose(
                                attnT[:, head, tsl:tsl + P], pvn[:])

            # ---------------- main pipeline ----------------
            for t in range(NT):
                # stream hidden chunk t (32 h-slices, reused by 3 passes)
                hts = []
                for h in range(NHC):
                    ht = hsp.tile([P, TC], bf16, tag="ht",
                                  name=f"ht_{t}_{h}")
                    nc.scalar.dma_start(
                        ht[:], hidT[h * P:(h + 1) * P,
                                    t * TC:(t + 1) * TC])
                    hts.append(ht)
                # 7 sequential accumulation groups per chunk:
                # c=0..4 (q heads + k, moving ht) then v s=0,1 (ht
                # stationary, natural layout).  Sequential groups may
                # share PSUM banks; only interleaved ones may not.
                for c in range(5):
                    ps = aps.tile([P, TC], f32, tag="aps",
                                  name=f"aps_{t}_{c}")
                    for h in range(NHC):
                        nc.tensor.matmul(
                            ps[:], wq[:, h, c * P:(c + 1) * P], hts[h][:],
                            start=(h == 0), stop=(h == NHC - 1))
                    xf = xfp.tile([P, TC], bf16, tag="xf",
                                  name=f"xf_{t}_{c}")
                    nc.scalar.copy(xf[:], ps[:])
                    rope(c, t, xf)
                for s in range(2):
                    ps = aps.tile([P, TC], f32, tag="aps",
                                  name=f"aps_{t}_v{s}")
                    for h in range(NHC):
                        nc.tensor.matmul(
                            ps[:, 0:P],
                            hts[h][:, s * P:(s + 1) * P],
                            wq[:, h, 640:768],
                            start=(h == 0), stop=(h == NHC - 1))
                    nc.scalar.copy(vnat[:, 2 * t + s, 0:P], ps[:, 0:P])
                if t % 2 == 1:
                    g = (t - 1) // 2
                    attn_block(g)
                    wo_queue.extend(
                        [(4 * g + i, oc) for i in range(4)
                         for oc in range(8)])
            # drain the last block's WO groups
            while wo_queue:
                emit_wo_group()
            if DEBUG_DUMP:
                nc.sync.dma_start(dbg_qkT[:], qkT[:])
                nc.sync.dma_start(dbg_vnat[:], vnat[:])
                nc.sync.dma_start(dbg_attnT[:], attnT[:])

    nc.compile()
    return nc


def _rope_tables(positions):
    pos = positions.astype(np.float64)
    inv_freq = 1.0 / (BASE ** (np.arange(HALF, dtype=np.float64) / HALF))
    freqs = pos[:, None] * inv_freq[None, :]          # [T, 64]
    cos = np.cos(freqs)
    sin = np.sin(freqs)
    cosT = np.concatenate([cos, cos], axis=1).T       # [128, T]
    sinT = np.concatenate([-sin, sin], axis=1).T      # sign folded
    return cosT, sinT


def kernel(positions, hidden_states, wqkv, wo):
    global _COMPILED
    if _COMPILED is None:
        _COMPILED = _build()
    nc = _COMPILED

    scale = HD ** -0.5
    cosT, sinT = _rope_tables(positions)
    cosq = np.ascontiguousarray(cosT * scale).astype(bfloat16)
    sinq = np.ascontiguousarray(sinT * scale).astype(bfloat16)
    cosk = np.ascontiguousarray(cosT).astype(bfloat16)
    sink = np.ascontiguousarray(sinT).astype(bfloat16)

    hidT = np.ascontiguousarray(np.asarray(hidden_states).T).astype(bfloat16)

    # causal triangle for the diagonal 128x128 sub-block, [k, q] layout
    kl = np.arange(P)[:, None]
    ql = np.arange(P)[None, :]
    maskd = np.where(kl <= ql, 0.0, NEG).astype(np.float32)

    wqkv = np.asarray(wqkv)
    wo = np.asarray(wo)
    in_maps = []
    for r in range(NCORES):
        qc = slice(r * QCOLS, (r + 1) * QCOLS)
        kc = slice(NH * HD + r * HD, NH * HD + (r + 1) * HD)
        vc = slice((NH + NKV) * HD + r * HD, (NH + NKV) * HD + (r + 1) * HD)
        wq_s = np.ascontiguousarray(np.concatenate(
            [wqkv[:, qc], wqkv[:, kc], wqkv[:, vc]], axis=1)).astype(bfloat16)
        wo_r = np.ascontiguousarray(
            wo[qc, :].reshape(QH, P, H).transpose(1, 0, 2)).astype(bfloat16)
        in_maps.append({
            "hidT": hidT, "wq_d": wq_s, "wo_d": wo_r,
            "cosq": cosq, "sinq": sinq, "cosk": cosk, "sink": sink,
            "maskd": maskd,
        })

    global _LAST_IN_MAPS
    _LAST_IN_MAPS = in_maps
    res = run_bass_kernel_spmd(nc, in_maps, list(range(NCORES)))
    out = res.results[0]["part"].astype(np.float64)
    for r in range(1, NCORES):
        out += res.results[r]["part"].astype(np.float64)
    return out.astype(np.float32)
